# revision 1
# baseline (speedup 1.0000x reference)
"""Trainium2 Bass kernel: Encoder_HieStackedCorr (UnCorrVmat_Detail, t_method='uncorr').

Math (per batch b):
    W1 = wn(U1_v, U1_g); W2 = wn(U2_v, U2_g)
    R = relu(V @ W1.T + b1)          [N, LR]
    L = relu(V @ W2.T + b2)          [N, LR]
    UnCorr = L @ R.T                 [N, N]
    d[n] = UnCorr[n, n] = sum_l L[n,l] R[n,l]
    dr = 1/sqrt(d + eps)
    A = 1 + I - dr dr^T * UnCorr
    v = mean_n (A @ V) = (1/N) * s @ V  where s[m] = N + 1 - dr[m] * (t . R[m,:]),
                                              t = sum_n dr[n] L[n,:]
    feat = v @ W_lin.T + b_lin ; out = batchnorm(feat)   (training-mode stats)

The N x N matrix is never materialized: mean-pooling commutes with the matmul,
collapsing the O(B N^2 (LR+D)) reference into O(B N D LR) work.

Sharding: data-parallel over batch, 4 batches per core on 8 cores.  Each core
computes v for its 4 batches; the tiny [32,256] linear + batchnorm epilogue
(needs cross-core batch stats) runs on host.
"""

import os
import sys

import numpy as np

for _p in ("/opt/trn_rl_repo", "/root/.axon_site/_ro/trn_rl_repo"):
    if os.path.isdir(_p) and _p not in sys.path:
        sys.path.insert(0, _p)
        break

import ml_dtypes  # noqa: E402
import concourse.bass as bass  # noqa: E402
import concourse.bacc as bacc  # noqa: E402
import concourse.mybir as mybir  # noqa: E402
import concourse.tile as tile  # noqa: E402
from concourse.bass_utils import run_bass_kernel_spmd  # noqa: E402


def _ensure_ntff_hook():
    """Shim the missing ``antenv.axon_hooks`` registry so trace=True works.

    The agent image's ``antenv`` lacks ``axon_hooks``; the ctypes NTFF driver
    in ``trn_agent_boot.trn_boot`` is present and the injected libaxon_pjrt.so
    exports the profile symbols, so wire them together here.
    """
    import types

    try:
        from antenv.axon_hooks import get_axon_ntff_profile_hook  # noqa: F401
        return
    except ImportError:
        pass
    try:
        from trn_agent_boot.trn_boot import _ntff_profile_via_ctypes
        hook = _ntff_profile_via_ctypes("/opt/axon/libaxon_pjrt.so")
    except Exception:
        hook = None
    mod = types.ModuleType("antenv.axon_hooks")
    mod._hook = hook
    mod.get_axon_ntff_profile_hook = lambda: mod._hook
    mod.set_axon_ntff_profile_hook = lambda h: setattr(mod, "_hook", h)
    sys.modules["antenv.axon_hooks"] = mod


_ensure_ntff_hook()

# Problem constants (hardcoded; see module docstring).
B, N, D, LR, EMB = 32, 2048, 256, 64, 256
NCORES = 8
B_LOC = B // NCORES          # 4 batches per core
ROWS = B_LOC * N             # 8192 rows per core
NT_B = N // 128              # 16 row-tiles per batch
NBLK = N // 512              # 4 512-row blocks per batch
EPS_DIAG = 1e-6
EPS_BN = 1e-5

F32 = mybir.dt.float32
BF16 = mybir.dt.bfloat16

# dt: dtype for V/Vt/W, the L/R activations and every matmul operand
# ("f32" = exact but 4 cyc/row on the PE, "bf16" = 1 cyc/row).
CONFIG = dict(dt="f32", trace=False)

_CACHE = {}


def _build(cfg):
    DT = BF16 if cfg["dt"] == "bf16" else F32
    nc = bacc.Bacc("TRN2", target_bir_lowering=False, debug=False)

    v_d = nc.dram_tensor("v", [ROWS, D], DT, kind="ExternalInput").ap()
    vt_d = nc.dram_tensor("vt", [2, 128, ROWS], DT, kind="ExternalInput").ap()
    w1_d = nc.dram_tensor("w1t", [2, 128, LR], DT, kind="ExternalInput").ap()
    w2_d = nc.dram_tensor("w2t", [2, 128, LR], DT, kind="ExternalInput").ap()
    b1_d = nc.dram_tensor("b1", [LR, 1], F32, kind="ExternalInput").ap()
    b2_d = nc.dram_tensor("b2", [LR, 1], F32, kind="ExternalInput").ap()
    out_d = nc.dram_tensor("vmean", [1, B_LOC * D], F32, kind="ExternalOutput").ap()

    with tile.TileContext(nc) as tc:
        with (
            tc.tile_pool(name="const", bufs=1) as cpool,
            tc.tile_pool(name="vst", bufs=1) as vpool,
            tc.tile_pool(name="lrbuf", bufs=2) as lrpool,
            tc.tile_pool(name="blk", bufs=3) as bpool,
            tc.tile_pool(name="rows", bufs=2) as rpool,
            tc.tile_pool(name="ps_lr", bufs=2, space="PSUM") as ps_lr,
            tc.tile_pool(name="ps_d", bufs=1, space="PSUM") as ps_d,
            tc.tile_pool(name="ps_misc", bufs=1, space="PSUM") as ps_misc,
            tc.tile_pool(name="dram", bufs=2, space="DRAM") as dpool,
        ):
            # ---- constants / weights ----
            w1_sb = cpool.tile([128, 2 * LR], DT)
            w2_sb = cpool.tile([128, 2 * LR], DT)
            nc.sync.dma_start(
                w1_sb[:].rearrange("p (c l) -> p c l", c=2),
                w1_d.rearrange("c p l -> p c l"),
            )
            nc.sync.dma_start(
                w2_sb[:].rearrange("p (c l) -> p c l", c=2),
                w2_d.rearrange("c p l -> p c l"),
            )
            b1_sb = cpool.tile([LR, 1], F32)
            b2_sb = cpool.tile([LR, 1], F32)
            nc.sync.dma_start(b1_sb[:], b1_d[:])
            nc.sync.dma_start(b2_sb[:], b2_d[:])
            ones64 = cpool.tile([LR, 1], DT)
            nc.vector.memset(ones64[:], 1.0)
            ones_k1 = cpool.tile([1, LR], DT)
            nc.vector.memset(ones_k1[:], 1.0)
            eps_sb = cpool.tile([1, 1], F32)
            nc.vector.memset(eps_sb[:], EPS_DIAG)

            out_sb = cpool.tile([1, B_LOC * D], F32)

            # per-batch persistent tiles, double buffered across batches
            v_t = {}
            vt_t = {}
            for b in range(B_LOC):
                # natural V for this batch: tile j at cols [j*D, (j+1)*D)
                v_t[b] = vpool.tile([128, NT_B * D], DT, tag="vnat", name=f"vnat{b}")
                # transposed V, both d-chunks: chunk c at cols [c*N, (c+1)*N)
                vt_t[b] = vpool.tile([128, 2 * N], DT, tag="vt", name=f"vt{b}")
                src = v_d.rearrange("(t p) d -> p t d", p=128)
                nc.sync.dma_start(
                    v_t[b][:].rearrange("p (t d) -> p t d", t=NT_B),
                    src[:, b * NT_B:(b + 1) * NT_B, :],
                )
                nc.sync.dma_start(
                    vt_t[b][:].rearrange("p (c n) -> p c n", c=2),
                    vt_d[:, :, b * N:(b + 1) * N].rearrange("c p n -> p c n"),
                )

            for b in range(B_LOC):
                L_sb = lrpool.tile([LR, N], DT, tag="L")
                R_sb = lrpool.tile([LR, N], DT, tag="R")
                sq_row = rpool.tile([1, N], F32, tag="sq")     # sqrt(d + eps)
                dr_row = rpool.tile([1, N], F32, tag="dr")     # 1/sqrt(d + eps)
                s_row = rpool.tile([1, N], F32, tag="s")       # ((N+1) - c)/N
                for blk in range(NBLK):
                    f0 = blk * 512
                    # L/R = V @ W.T in transposed layout [LR, n-block]
                    L_ps = ps_lr.tile([LR, 512], F32, tag="Lps")
                    R_ps = ps_lr.tile([LR, 512], F32, tag="Rps")
                    for c in range(2):
                        rhs = vt_t[b][:, c * N + f0:c * N + f0 + 512]
                        nc.tensor.matmul(
                            L_ps[:], w2_sb[:, c * LR:(c + 1) * LR], rhs,
                            start=(c == 0), stop=(c == 1),
                        )
                        nc.tensor.matmul(
                            R_ps[:], w1_sb[:, c * LR:(c + 1) * LR], rhs,
                            start=(c == 0), stop=(c == 1),
                        )
                    # relu(+bias): R on ACT, L on DVE (balance engines)
                    nc.scalar.activation(
                        R_sb[:, f0:f0 + 512], R_ps[:],
                        mybir.ActivationFunctionType.Relu, bias=b1_sb[:], scale=1.0,
                    )
                    nc.vector.tensor_scalar(
                        L_sb[:, f0:f0 + 512], L_ps[:], b2_sb[:], 0.0,
                        mybir.AluOpType.add, mybir.AluOpType.max,
                    )
                    # diag: d[n] = sum_l L[l,n]*R[l,n] -> [1,512] via ones-matmul
                    prod = bpool.tile([LR, 512], DT, tag="prod")
                    nc.vector.tensor_tensor(
                        prod[:], L_sb[:, f0:f0 + 512], R_sb[:, f0:f0 + 512],
                        mybir.AluOpType.mult,
                    )
                    d_ps = ps_d.tile([1, 512], F32, tag="dps")
                    nc.tensor.matmul(
                        d_ps[:], ones64[:], prod[:],
                        start=True, stop=True,
                    )
                    # dr = 1/sqrt(d + eps)  (Rsqrt is banned on ACT; DVE reciprocal)
                    nc.scalar.activation(
                        sq_row[:, f0:f0 + 512], d_ps[:],
                        mybir.ActivationFunctionType.Sqrt, bias=eps_sb[:], scale=1.0,
                    )
                    nc.vector.reciprocal(
                        dr_row[:, f0:f0 + 512], sq_row[:, f0:f0 + 512]
                    )

                # t = sum_n dr[n] * L[n,:]   (chained fused multiply-reduce)
                dr_dt = dr_row
                if DT != F32:
                    dr_dt = rpool.tile([1, N], DT, tag="dr_dt", name=f"drdt{b}")
                    nc.scalar.activation(
                        dr_dt[:], dr_row[:], mybir.ActivationFunctionType.Copy
                    )
                ldr = lrpool.tile([LR, N], DT, tag="ldr", name=f"ldr{b}")
                for blk in range(NBLK):
                    f0 = blk * 512
                    rep_ps = ps_misc.tile([LR, 512], F32, tag="rep")
                    nc.tensor.matmul(
                        rep_ps[:], ones_k1[:], dr_dt[:, f0:f0 + 512],
                        start=True, stop=True,
                    )
                    nc.vector.tensor_tensor(
                        ldr[:, f0:f0 + 512], L_sb[:, f0:f0 + 512], rep_ps[:],
                        mybir.AluOpType.mult,
                    )
                t_sb = bpool.tile([LR, 1], F32, tag="t", name=f"tacc{b}")
                nc.vector.tensor_reduce(
                    t_sb[:], ldr[:], mybir.AxisListType.X, mybir.AluOpType.add,
                )
                t_dt = t_sb
                if DT != F32:
                    t_dt = bpool.tile([LR, 1], DT, tag="t_dt", name=f"tdt{b}")
                    nc.scalar.activation(
                        t_dt[:], t_sb[:], mybir.ActivationFunctionType.Copy
                    )

                # u = t . R[m,:] -> [1,512] blocks; c = u / sq; s = ((N+1)-c)/N
                for blk in range(NBLK):
                    f0 = blk * 512
                    u_ps = ps_misc.tile([1, 512], F32, tag="ups")
                    nc.tensor.matmul(
                        u_ps[:], t_dt[:], R_sb[:, f0:f0 + 512],
                        start=True, stop=True,
                    )
                    c_row = bpool.tile([1, 512], F32, tag="crow")
                    nc.vector.tensor_tensor(
                        c_row[:], u_ps[:], dr_row[:, f0:f0 + 512],
                        mybir.AluOpType.mult,
                    )
                    nc.scalar.activation(
                        s_row[:, f0:f0 + 512], c_row[:],
                        mybir.ActivationFunctionType.Copy,
                        bias=float(N + 1) / N, scale=-1.0 / N,
                    )

                # scatter s to partitions: s_col[p, j] = s[j*128 + p].
                # A direct SBUF->SBUF rearrange is NOT usable: the source AP's
                # first dim is interpreted as physical partitions by the DMA
                # descriptor generator (HW reads partitions 1.. as garbage).
                # Bounce through DRAM, where APs are plain strided views.
                s_dram = dpool.tile([1, N], F32, tag="sdram", name=f"sdram{b}")
                nc.sync.dma_start(s_dram[:], s_row[:])
                s_col = bpool.tile([128, NT_B], F32, tag="scol")
                nc.sync.dma_start(
                    s_col[:], s_dram.rearrange("a (j p) -> (a p) j", p=128)
                )
                s_dt = s_col
                if DT != F32:
                    s_dt = bpool.tile([128, NT_B], DT, tag="scol_dt")
                    nc.scalar.activation(
                        s_dt[:], s_col[:], mybir.ActivationFunctionType.Copy
                    )

                # v_mean = s^T @ V  (accumulate over the 16 row-tiles)
                v_ps = ps_misc.tile([1, D], F32, tag="vps")
                for j in range(NT_B):
                    nc.tensor.matmul(
                        v_ps[:], s_dt[:, j:j + 1],
                        v_t[b][:, j * D:(j + 1) * D],
                        start=(j == 0), stop=(j == NT_B - 1),
                    )
                nc.scalar.activation(
                    out_sb[:, b * D:(b + 1) * D], v_ps[:],
                    mybir.ActivationFunctionType.Copy,
                )

            nc.sync.dma_start(out_d[:], out_sb[:])

    nc.compile()
    return nc


def _host_prep(inputs, cfg):
    """Weight-norm, transposes, casts; returns per-core input maps + epilogue data."""
    np_dt = ml_dtypes.bfloat16 if cfg["dt"] == "bf16" else np.float32

    def wn(v, g):
        return v * (g / np.linalg.norm(v.astype(np.float64), axis=1)).astype(
            np.float32
        )[:, None]

    W1 = wn(np.asarray(inputs["U1_v"], np.float32), np.asarray(inputs["U1_g"], np.float32))
    W2 = wn(np.asarray(inputs["U2_v"], np.float32), np.asarray(inputs["U2_g"], np.float32))
    w1t = np.ascontiguousarray(W1.T).reshape(2, 128, LR).astype(np_dt)
    w2t = np.ascontiguousarray(W2.T).reshape(2, 128, LR).astype(np_dt)
    b1 = np.asarray(inputs["U1_b"], np.float32).reshape(LR, 1)
    b2 = np.asarray(inputs["U2_b"], np.float32).reshape(LR, 1)

    V = np.asarray(inputs["Vmat"], np.float32)  # [B, N, D]
    in_maps = []
    for k in range(NCORES):
        Vk = np.ascontiguousarray(V[k * B_LOC:(k + 1) * B_LOC].reshape(ROWS, D))
        vt = np.ascontiguousarray(Vk.T).reshape(2, 128, ROWS).astype(np_dt)
        in_maps.append({
            "v": Vk.astype(np_dt),
            "vt": vt,
            "w1t": w1t,
            "w2t": w2t,
            "b1": b1,
            "b2": b2,
        })
    return in_maps


def _epilogue(v_mean, inputs):
    """feat = v_mean @ W_lin.T + b_lin, then training-mode batchnorm."""
    W_lin = np.asarray(inputs["W_lin"], np.float32)
    b_lin = np.asarray(inputs["b_lin"], np.float32)
    gamma = np.asarray(inputs["gamma"], np.float32)
    beta = np.asarray(inputs["beta"], np.float32)
    feat = v_mean.astype(np.float32) @ W_lin.T + b_lin
    mu = feat.mean(axis=0)
    var = feat.var(axis=0)
    out = (feat - mu) / np.sqrt(var + EPS_BN) * gamma + beta
    return out.astype(np.float32)


def kernel(**inputs):
    cfg = dict(CONFIG)
    key = (cfg["dt"],)
    if key not in _CACHE:
        _CACHE[key] = _build(cfg)
    nc = _CACHE[key]
    in_maps = _host_prep(inputs, cfg)
    res = run_bass_kernel_spmd(
        nc, in_maps, core_ids=list(range(NCORES)), trace=cfg["trace"]
    )
    kernel.last_results = res
    v_mean = np.concatenate(
        [res.results[k]["vmean"].reshape(B_LOC, D) for k in range(NCORES)], axis=0
    )
    return _epilogue(v_mean, inputs)



# revision 5
# speedup vs baseline: 1.4485x; 1.4485x over previous
"""Trainium2 Bass kernel: Encoder_HieStackedCorr (UnCorrVmat_Detail, t_method='uncorr').

Math (per batch b):
    W1 = wn(U1_v, U1_g); W2 = wn(U2_v, U2_g)
    R = relu(V @ W1.T + b1)          [N, LR]
    L = relu(V @ W2.T + b2)          [N, LR]
    UnCorr = L @ R.T                 [N, N]
    d[n] = UnCorr[n, n] = sum_l L[n,l] R[n,l]
    dr = 1/sqrt(d + eps)
    A = 1 + I - dr dr^T * UnCorr
    v = mean_n (A @ V) = (1/N) * s @ V  where s[m] = N + 1 - dr[m] * (t . R[m,:]),
                                              t = sum_n dr[n] L[n,:]
    feat = v @ W_lin.T + b_lin ; out = batchnorm(feat)   (training-mode stats)

The N x N matrix is never materialized: mean-pooling commutes with the matmul,
collapsing the O(B N^2 (LR+D)) reference into O(B N D LR) work.

Kernel structure (v2):
  - W1 || W2 stacked into one [128, 128] lhsT: a single matmul pair per
    512-block produces both R (partitions 0-63) and L (64-127); one fused
    relu+bias on the whole [128, 512] PSUM tile.
  - Diag via ones[64,64] lhsT: d arrives already broadcast over 64
    partitions ([64, 512] PSUM), so rsqrt runs wide (ACT sqrt + DVE
    reciprocal_approx_fast) and no separate broadcast matmul is needed.
  - t via chained tensor_tensor_reduce (mult+add accumulate across blocks).
  - dr is folded into R (Rd = R * drrep on GpSimd), so the u-matmul yields
    c = dr*u directly and s = (N+1-c)/N comes from one ACT affine per block.
  - s scattered to column layout via a DRAM bounce, then 16 accumulating
    matmuls against natural-layout V give v_mean.

Sharding: data-parallel over batch, 4 batches per core on 8 cores.  The tiny
[32,256] linear + batchnorm epilogue (cross-core batch stats) runs on host.
"""

import os
import sys

import numpy as np

for _p in ("/opt/trn_rl_repo", "/root/.axon_site/_ro/trn_rl_repo"):
    if os.path.isdir(_p) and _p not in sys.path:
        sys.path.insert(0, _p)
        break

import ml_dtypes  # noqa: E402
import concourse.bass as bass  # noqa: E402
import concourse.bacc as bacc  # noqa: E402
import concourse.mybir as mybir  # noqa: E402
import concourse.tile as tile  # noqa: E402
from concourse.bass_utils import run_bass_kernel_spmd  # noqa: E402


def _ensure_ntff_hook():
    """Shim the missing ``antenv.axon_hooks`` registry so trace=True works."""
    import types

    try:
        from antenv.axon_hooks import get_axon_ntff_profile_hook  # noqa: F401
        return
    except ImportError:
        pass
    try:
        from trn_agent_boot.trn_boot import _ntff_profile_via_ctypes
        hook = _ntff_profile_via_ctypes("/opt/axon/libaxon_pjrt.so")
    except Exception:
        hook = None
    mod = types.ModuleType("antenv.axon_hooks")
    mod._hook = hook
    mod.get_axon_ntff_profile_hook = lambda: mod._hook
    mod.set_axon_ntff_profile_hook = lambda h: setattr(mod, "_hook", h)
    sys.modules["antenv.axon_hooks"] = mod


_ensure_ntff_hook()

# Problem constants (hardcoded; see module docstring).
B, N, D, LR, EMB = 32, 2048, 256, 64, 256
NCORES = 8
B_LOC = B // NCORES          # 4 batches per core
NT_B = N // 128              # 16 row-tiles per batch
NBLK = N // 512              # 4 512-col blocks per batch
VIN_W = 2 * N + NT_B * D     # per-batch packed width: vt (2*N) + v-nat (16*D)
EPS_DIAG = 1e-6
EPS_BN = 1e-5

F32 = mybir.dt.float32
BF16 = mybir.dt.bfloat16

CONFIG = dict(dt="bf16", trace=False)

_CACHE = {}


def _build(cfg):
    nc = bacc.Bacc("TRN2", target_bir_lowering=False, debug=False)

    vin_d = nc.dram_tensor("vin", [128, B_LOC, VIN_W], BF16, kind="ExternalInput").ap()
    w12_d = nc.dram_tensor("w12", [128, 2, 128], BF16, kind="ExternalInput").ap()
    b12_d = nc.dram_tensor("b12", [128, 1], F32, kind="ExternalInput").ap()
    out_d = nc.dram_tensor("vmean", [1, B_LOC * D], F32, kind="ExternalOutput").ap()

    AF = mybir.ActivationFunctionType
    OP = mybir.AluOpType

    with tile.TileContext(nc) as tc:
        with (
            tc.tile_pool(name="const", bufs=1) as cpool,
            tc.tile_pool(name="vin", bufs=B_LOC) as vpool,
            tc.tile_pool(name="lr", bufs=2) as lrpool,
            tc.tile_pool(name="rd", bufs=2) as rdpool,
            tc.tile_pool(name="blk", bufs=3) as bpool,
            tc.tile_pool(name="sq", bufs=3) as sqpool,
            tc.tile_pool(name="dr", bufs=3) as drpool,
            tc.tile_pool(name="tsm", bufs=12) as tpool,
            tc.tile_pool(name="srow", bufs=2) as srpool,
            tc.tile_pool(name="scol", bufs=2) as scpool,
            tc.tile_pool(name="ps_lr", bufs=2, space="PSUM") as ps_lr,
            tc.tile_pool(name="ps_d", bufs=2, space="PSUM") as ps_d,
            tc.tile_pool(name="ps_u", bufs=2, space="PSUM") as ps_u,
            tc.tile_pool(name="ps_v", bufs=2, space="PSUM") as ps_v,
            tc.tile_pool(name="dram", bufs=2, space="DRAM") as dpool,
        ):
            # ---- constants / weights ----
            w12_sb = cpool.tile([128, 2 * 128], BF16)
            nc.sync.dma_start(
                w12_sb[:].rearrange("p (c m) -> p c m", c=2), w12_d[:]
            )
            b12_sb = cpool.tile([128, 1], F32)
            nc.sync.dma_start(b12_sb[:], b12_d[:])
            ones64 = cpool.tile([64, 64], BF16)
            nc.vector.memset(ones64[:], 1.0)
            eps64 = cpool.tile([64, 1], F32)
            nc.vector.memset(eps64[:], EPS_DIAG)
            out_sb = cpool.tile([1, B_LOC * D], F32)

            # ---- per-batch inputs: one packed 2MB DMA each ----
            vin = {}
            for b in range(B_LOC):
                vin[b] = vpool.tile([128, VIN_W], BF16, tag="vin", name=f"vin{b}")
                nc.sync.dma_start(vin[b][:], vin_d[:, b, :])

            for b in range(B_LOC):
                vt_v = vin[b][:, 0:2 * N]          # [128, c*N+n] transposed V
                vnat = vin[b][:, 2 * N:]           # [128, j*D+dd] natural V

                R_sb = lrpool.tile([64, N], BF16, tag="R", name=f"R{b}")
                L_sb = lrpool.tile([64, N], BF16, tag="L", name=f"L{b}")
                Rd_sb = rdpool.tile([64, N], BF16, tag="Rd")
                ldr_sb = rdpool.tile([64, N], BF16, tag="ldr")

                for blk in range(NBLK):
                    f0 = blk * 512
                    # [R; L] = W12.T @ vt  (R rows 0-63, L rows 64-127)
                    lr_ps = ps_lr.tile([128, 512], F32, tag="lrps")
                    for c in range(2):
                        nc.tensor.matmul(
                            lr_ps[:],
                            w12_sb[:, c * 128:(c + 1) * 128],
                            vt_v[:, c * N + f0:c * N + f0 + 512],
                            start=(c == 0), stop=(c == 1),
                        )
                    # split relu: R half on ACT, L half on ACT (cross-base
                    # PSUM read is legal); both land at partition base 0.
                    nc.scalar.activation(
                        R_sb[:, f0:f0 + 512], lr_ps[0:64, :], AF.Relu,
                        bias=b12_sb[0:64, :], scale=1.0,
                    )
                    if blk % 2 == 0:
                        nc.scalar.activation(
                            L_sb[:, f0:f0 + 512], lr_ps[64:128, :], AF.Relu,
                            bias=b12_sb[64:128, :], scale=1.0,
                        )
                    else:
                        nc.vector.tensor_scalar(
                            L_sb[:, f0:f0 + 512], lr_ps[64:128, :],
                            b12_sb[64:128, :], 0.0, OP.add, OP.max,
                        )
                    # prod = L * R on GpSimd (SBUF-only engine)
                    prod = bpool.tile([64, 512], BF16, tag="prod")
                    nc.gpsimd.tensor_tensor(
                        prod[:], R_sb[:, f0:f0 + 512],
                        L_sb[:, f0:f0 + 512], OP.mult,
                    )
                    # d broadcast over 64 partitions via ones-lhsT matmul
                    d_ps = ps_d.tile([64, 512], F32, tag="dps")
                    nc.tensor.matmul(
                        d_ps[:], ones64[:], prod[:], start=True, stop=True,
                    )
                    # drrep = 1/sqrt(d + eps), computed wide
                    sqrep = sqpool.tile([64, 512], F32, tag="sq")
                    nc.scalar.activation(
                        sqrep[:], d_ps[:], AF.Sqrt, bias=eps64[:], scale=1.0,
                    )
                    drrep = drpool.tile([64, 512], F32, tag="dr")
                    nc.vector.reciprocal_approx_fast(drrep[:], sqrep[:])
                    # ldr = L * drrep  (summed below for t)
                    nc.gpsimd.tensor_tensor(
                        ldr_sb[:, f0:f0 + 512], L_sb[:, f0:f0 + 512],
                        drrep[:], OP.mult,
                    )
                    # Rd = R * drrep  (so u-matmul yields c = dr*u directly)
                    nc.gpsimd.tensor_tensor(
                        Rd_sb[:, f0:f0 + 512], R_sb[:, f0:f0 + 512],
                        drrep[:], OP.mult,
                    )

                # t = sum_n dr[n] * L[:, n]
                t_f = tpool.tile([64, 1], F32, tag="tf", name=f"tf{b}")
                nc.vector.tensor_reduce(
                    t_f[:], ldr_sb[:], mybir.AxisListType.X, OP.add,
                )
                t_bf = tpool.tile([64, 1], BF16, tag="tbf", name=f"tbf{b}")
                nc.scalar.activation(t_bf[:], t_f[:], AF.Copy)

                # c = dr * (t . R[m,:]) ; s = ((N+1) - c)/N
                s_row = srpool.tile([1, N], BF16, tag="srow")
                for blk in range(NBLK):
                    f0 = blk * 512
                    u_ps = ps_u.tile([1, 512], F32, tag="ups")
                    nc.tensor.matmul(
                        u_ps[:], t_bf[:], Rd_sb[:, f0:f0 + 512],
                        start=True, stop=True,
                    )
                    if blk % 2 == 0:
                        nc.scalar.activation(
                            s_row[:, f0:f0 + 512], u_ps[:], AF.Copy,
                            bias=float(N + 1) / N, scale=-1.0 / N,
                        )
                    else:
                        nc.vector.tensor_scalar(
                            s_row[:, f0:f0 + 512], u_ps[:],
                            -1.0 / N, float(N + 1) / N, OP.mult, OP.add,
                        )

                # scatter s to partitions via DRAM bounce (SBUF->SBUF partition
                # rearrange is not expressible; DRAM APs are plain strided).
                s_dram = dpool.tile([1, N], BF16, tag="sdram", name=f"sdram{b}")
                nc.sync.dma_start(s_dram[:], s_row[:])
                s_col = scpool.tile([128, NT_B], BF16, tag="scol")
                nc.sync.dma_start(
                    s_col[:], s_dram.rearrange("a (j p) -> (a p) j", p=128)
                )

                # v_mean = s^T @ V  (accumulate over the 16 row-tiles)
                v_ps = ps_v.tile([1, D], F32, tag="vps")
                for j in range(NT_B):
                    nc.tensor.matmul(
                        v_ps[:], s_col[:, j:j + 1],
                        vnat[:, j * D:(j + 1) * D],
                        start=(j == 0), stop=(j == NT_B - 1),
                    )
                nc.scalar.activation(
                    out_sb[:, b * D:(b + 1) * D], v_ps[:], AF.Copy,
                )

            nc.sync.dma_start(out_d[:], out_sb[:])

    nc.compile()
    return nc


def _host_prep(inputs, cfg):
    """Weight-norm, packing, casts; returns per-core input maps."""
    def wn(v, g):
        return v * (g / np.linalg.norm(v.astype(np.float64), axis=1)).astype(
            np.float32
        )[:, None]

    W1 = wn(np.asarray(inputs["U1_v"], np.float32), np.asarray(inputs["U1_g"], np.float32))
    W2 = wn(np.asarray(inputs["U2_v"], np.float32), np.asarray(inputs["U2_g"], np.float32))
    # lhsT layout [d, m]: m 0-63 -> R (W1), 64-127 -> L (W2); split d in 2 chunks
    W12T = np.concatenate([W1.T, W2.T], axis=1)          # [D, 128]
    w12 = np.ascontiguousarray(
        W12T.reshape(2, 128, 128)
    ).astype(ml_dtypes.bfloat16)                          # [c, d, m]
    w12 = np.ascontiguousarray(w12.transpose(1, 0, 2))    # [d, c, m]
    b12 = np.concatenate([
        np.asarray(inputs["U1_b"], np.float32),
        np.asarray(inputs["U2_b"], np.float32),
    ]).reshape(128, 1)

    V = np.asarray(inputs["Vmat"], np.float32)  # [B, N, D]
    Vb = V.astype(ml_dtypes.bfloat16)
    in_maps = []
    for k in range(NCORES):
        packs = []
        for b in range(B_LOC):
            Vk = Vb[k * B_LOC + b]                                    # [N, D]
            vt = Vk.T.reshape(2, 128, N).transpose(1, 0, 2).reshape(128, 2 * N)
            vn = Vk.reshape(NT_B, 128, D).transpose(1, 0, 2).reshape(128, NT_B * D)
            packs.append(np.concatenate([vt, vn], axis=1))            # [128, VIN_W]
        vin = np.ascontiguousarray(np.stack(packs, axis=1))           # [128, B_LOC, VIN_W]
        in_maps.append({"vin": vin, "w12": w12, "b12": b12})
    return in_maps


def _epilogue(v_mean, inputs):
    """feat = v_mean @ W_lin.T + b_lin, then training-mode batchnorm."""
    W_lin = np.asarray(inputs["W_lin"], np.float32)
    b_lin = np.asarray(inputs["b_lin"], np.float32)
    gamma = np.asarray(inputs["gamma"], np.float32)
    beta = np.asarray(inputs["beta"], np.float32)
    feat = v_mean.astype(np.float32) @ W_lin.T + b_lin
    mu = feat.mean(axis=0)
    var = feat.var(axis=0)
    out = (feat - mu) / np.sqrt(var + EPS_BN) * gamma + beta
    return out.astype(np.float32)


def kernel(**inputs):
    cfg = dict(CONFIG)
    key = ("v2",)
    if key not in _CACHE:
        _CACHE[key] = _build(cfg)
    nc = _CACHE[key]
    in_maps = _host_prep(inputs, cfg)
    res = run_bass_kernel_spmd(
        nc, in_maps, core_ids=list(range(NCORES)), trace=cfg["trace"]
    )
    kernel.last_results = res
    v_mean = np.concatenate(
        [res.results[k]["vmean"].reshape(B_LOC, D) for k in range(NCORES)], axis=0
    )
    return _epilogue(v_mean, inputs)


# revision 10
# speedup vs baseline: 1.5805x; 1.0912x over previous
"""Trainium2 Bass kernel: Encoder_HieStackedCorr (UnCorrVmat_Detail, t_method='uncorr').

Math (per batch b):
    W1 = wn(U1_v, U1_g); W2 = wn(U2_v, U2_g)
    R = relu(V @ W1.T + b1)          [N, LR]
    L = relu(V @ W2.T + b2)          [N, LR]
    UnCorr = L @ R.T                 [N, N]
    d[n] = UnCorr[n, n] = sum_l L[n,l] R[n,l]
    dr = 1/sqrt(d + eps)
    A = 1 + I - dr dr^T * UnCorr
    v = mean_n (A @ V) = (1/N) * s @ V  where s[m] = N + 1 - dr[m] * (t . R[m,:]),
                                              t = sum_n dr[n] L[n,:]
    feat = v @ W_lin.T + b_lin ; out = batchnorm(feat)   (training-mode stats)

The N x N matrix is never materialized: mean-pooling commutes with the matmul,
collapsing the O(B N^2 (LR+D)) reference into O(B N D LR) work.

Kernel structure (v2):
  - W1 || W2 stacked into one [128, 128] lhsT: a single matmul pair per
    512-block produces both R (partitions 0-63) and L (64-127); one fused
    relu+bias on the whole [128, 512] PSUM tile.
  - Diag via ones[64,64] lhsT: d arrives already broadcast over 64
    partitions ([64, 512] PSUM), so rsqrt runs wide (ACT sqrt + DVE
    reciprocal_approx_fast) and no separate broadcast matmul is needed.
  - t via chained tensor_tensor_reduce (mult+add accumulate across blocks).
  - dr is folded into R (Rd = R * drrep on GpSimd), so the u-matmul yields
    c = dr*u directly and s = (N+1-c)/N comes from one ACT affine per block.
  - s scattered to column layout via a DRAM bounce, then 16 accumulating
    matmuls against natural-layout V give v_mean.

Sharding: data-parallel over batch, 4 batches per core on 8 cores.  The tiny
[32,256] linear + batchnorm epilogue (cross-core batch stats) runs on host.
"""

import os
import sys

import numpy as np

for _p in ("/opt/trn_rl_repo", "/root/.axon_site/_ro/trn_rl_repo"):
    if os.path.isdir(_p) and _p not in sys.path:
        sys.path.insert(0, _p)
        break

import ml_dtypes  # noqa: E402
import concourse.bass as bass  # noqa: E402
import concourse.bacc as bacc  # noqa: E402
import concourse.mybir as mybir  # noqa: E402
import concourse.tile as tile  # noqa: E402
from concourse.bass_utils import run_bass_kernel_spmd  # noqa: E402


def _ensure_ntff_hook():
    """Shim the missing ``antenv.axon_hooks`` registry so trace=True works."""
    import types

    try:
        from antenv.axon_hooks import get_axon_ntff_profile_hook  # noqa: F401
        return
    except ImportError:
        pass
    try:
        from trn_agent_boot.trn_boot import _ntff_profile_via_ctypes
        hook = _ntff_profile_via_ctypes("/opt/axon/libaxon_pjrt.so")
    except Exception:
        hook = None
    mod = types.ModuleType("antenv.axon_hooks")
    mod._hook = hook
    mod.get_axon_ntff_profile_hook = lambda: mod._hook
    mod.set_axon_ntff_profile_hook = lambda h: setattr(mod, "_hook", h)
    sys.modules["antenv.axon_hooks"] = mod


_ensure_ntff_hook()

# Problem constants (hardcoded; see module docstring).
B, N, D, LR, EMB = 32, 2048, 256, 64, 256
NCORES = 8
B_LOC = B // NCORES          # 4 batches per core
NT_B = N // 128              # 16 row-tiles per batch
NBLK = N // 512              # 4 512-col blocks per batch
VIN_W = 2 * N + NT_B * D     # per-batch packed width: vt (2*N) + v-nat (16*D)
EPS_DIAG = 1e-6
EPS_BN = 1e-5

F32 = mybir.dt.float32
BF16 = mybir.dt.bfloat16

CONFIG = dict(dt="bf16", trace=False)

_CACHE = {}


def _build(cfg):
    nc = bacc.Bacc("TRN2", target_bir_lowering=False, debug=False)

    vin_d = nc.dram_tensor("vin", [128, B_LOC, VIN_W], BF16, kind="ExternalInput").ap()
    w12_d = nc.dram_tensor("w12", [128, 2, 128], BF16, kind="ExternalInput").ap()
    b12_d = nc.dram_tensor("b12", [128, 1], F32, kind="ExternalInput").ap()
    out_d = nc.dram_tensor("vmean", [1, B_LOC * D], F32, kind="ExternalOutput").ap()

    AF = mybir.ActivationFunctionType
    OP = mybir.AluOpType

    with tile.TileContext(nc) as tc:
        with (
            tc.tile_pool(name="const", bufs=1) as cpool,
            tc.tile_pool(name="vin", bufs=B_LOC) as vpool,
            tc.tile_pool(name="lr", bufs=2) as lrpool,
            tc.tile_pool(name="rd", bufs=2) as rdpool,
            tc.tile_pool(name="blk", bufs=3) as bpool,
            tc.tile_pool(name="sq", bufs=3) as sqpool,
            tc.tile_pool(name="dr", bufs=3) as drpool,
            tc.tile_pool(name="tsm", bufs=12) as tpool,
            tc.tile_pool(name="srow", bufs=2) as srpool,
            tc.tile_pool(name="scol", bufs=2) as scpool,
            tc.tile_pool(name="ps_lr", bufs=3, space="PSUM") as ps_lr,
            tc.tile_pool(name="ps_d", bufs=2, space="PSUM") as ps_d,
            tc.tile_pool(name="ps_u", bufs=2, space="PSUM") as ps_u,
            tc.tile_pool(name="ps_v", bufs=1, space="PSUM") as ps_v,
            tc.tile_pool(name="dram", bufs=4, space="DRAM") as dpool,
        ):
            # ---- constants / weights ----
            w12_sb = cpool.tile([128, 2 * 128], BF16)
            nc.sync.dma_start(
                w12_sb[:].rearrange("p (c m) -> p c m", c=2), w12_d[:]
            )
            b12_sb = cpool.tile([128, 1], F32)
            nc.sync.dma_start(b12_sb[:], b12_d[:])
            ones64 = cpool.tile([64, 64], BF16)
            nc.vector.memset(ones64[:], 1.0)
            eps64 = cpool.tile([64, 1], F32)
            nc.vector.memset(eps64[:], EPS_DIAG)
            out_sb = cpool.tile([1, B_LOC * D], F32)

            # ---- per-batch inputs: vt halves first (feed main matmuls),
            # natural-V halves after (only needed by the final matmuls) ----
            vin = {}
            for b in range(B_LOC):
                vin[b] = vpool.tile([128, VIN_W], BF16, tag="vin", name=f"vin{b}")
                nc.sync.dma_start(vin[b][:, 0:2 * N], vin_d[:, b, 0:2 * N])
            for b in range(B_LOC):
                nc.sync.dma_start(vin[b][:, 2 * N:], vin_d[:, b, 2 * N:])

            for b in range(B_LOC):
                vt_v = vin[b][:, 0:2 * N]          # [128, c*N+n] transposed V
                vnat = vin[b][:, 2 * N:]           # [128, j*D+dd] natural V

                R_sb = lrpool.tile([64, N], BF16, tag="R", name=f"R{b}")
                L_sb = lrpool.tile([64, N], BF16, tag="L", name=f"L{b}")
                Rd_sb = rdpool.tile([64, N], BF16, tag="Rd")
                ldr_sb = rdpool.tile([64, N], BF16, tag="ldr")
                t_parts = tpool.tile([64, NBLK], F32, tag="tp", name=f"tp{b}")

                for blk in range(NBLK):
                    f0 = blk * 512
                    # [R; L] = W12.T @ vt  (R rows 0-63, L rows 64-127)
                    lr_ps = ps_lr.tile([128, 512], F32, tag="lrps")
                    for c in range(2):
                        nc.tensor.matmul(
                            lr_ps[:],
                            w12_sb[:, c * 128:(c + 1) * 128],
                            vt_v[:, c * N + f0:c * N + f0 + 512],
                            start=(c == 0), stop=(c == 1),
                        )
                    # split relu: R half on ACT, L half on ACT (cross-base
                    # PSUM read is legal); both land at partition base 0.
                    nc.scalar.activation(
                        R_sb[:, f0:f0 + 512], lr_ps[0:64, :], AF.Relu,
                        bias=b12_sb[0:64, :], scale=1.0,
                    )
                    if blk % 2 == 0:
                        nc.scalar.activation(
                            L_sb[:, f0:f0 + 512], lr_ps[64:128, :], AF.Relu,
                            bias=b12_sb[64:128, :], scale=1.0,
                        )
                    else:
                        nc.vector.tensor_scalar(
                            L_sb[:, f0:f0 + 512], lr_ps[64:128, :],
                            b12_sb[64:128, :], 0.0, OP.add, OP.max,
                        )
                    # prod = L * R on DVE (block-chain critical: feeds diag)
                    prod = bpool.tile([64, 512], BF16, tag="prod")
                    nc.vector.tensor_tensor(
                        prod[:], R_sb[:, f0:f0 + 512],
                        L_sb[:, f0:f0 + 512], OP.mult,
                    )
                    # d broadcast over 64 partitions via ones-lhsT matmul
                    d_ps = ps_d.tile([64, 512], F32, tag="dps")
                    nc.tensor.matmul(
                        d_ps[:], ones64[:], prod[:], start=True, stop=True,
                    )
                    # drrep = 1/sqrt(d + eps), computed wide
                    sqrep = sqpool.tile([64, 512], F32, tag="sq")
                    nc.scalar.activation(
                        sqrep[:], d_ps[:], AF.Sqrt, bias=eps64[:], scale=1.0,
                    )
                    drrep = drpool.tile([64, 512], F32, tag="dr")
                    nc.vector.reciprocal_approx_fast(drrep[:], sqrep[:])
                    # ldr = L * drrep (summed below for t); Rd = R * drrep so
                    # the u-matmul yields c = dr*u directly.  Rd has slack
                    # (phase B waits on t anyway) -> idle GpSimd; ldr of the
                    # last block gates t -> keep that one on DVE.
                    if blk == NBLK - 1:
                        nc.vector.tensor_tensor(
                            ldr_sb[:, f0:f0 + 512], L_sb[:, f0:f0 + 512],
                            drrep[:], OP.mult,
                        )
                    else:
                        nc.gpsimd.tensor_tensor(
                            ldr_sb[:, f0:f0 + 512], L_sb[:, f0:f0 + 512],
                            drrep[:], OP.mult,
                        )
                    nc.gpsimd.tensor_tensor(
                        Rd_sb[:, f0:f0 + 512], R_sb[:, f0:f0 + 512],
                        drrep[:], OP.mult,
                    )
                    # partial t for this block (full t = sum of partials)
                    nc.vector.tensor_reduce(
                        t_parts[:, blk:blk + 1], ldr_sb[:, f0:f0 + 512],
                        mybir.AxisListType.X, OP.add,
                    )

                # t = sum_n dr[n] * L[:, n]
                t_f = tpool.tile([64, 1], F32, tag="tf", name=f"tf{b}")
                nc.vector.tensor_reduce(
                    t_f[:], t_parts[:], mybir.AxisListType.X, OP.add,
                )
                t_bf = tpool.tile([64, 1], BF16, tag="tbf", name=f"tbf{b}")
                nc.scalar.activation(t_bf[:], t_f[:], AF.Copy)

                # c = dr * (t . R[m,:]) ; s = ((N+1) - c)/N.  The partition
                # scatter bounces through DRAM per 512-block so the strided
                # gather DMAs and the final matmul groups pipeline.
                s_row = srpool.tile([1, N], BF16, tag="srow")
                s_col = scpool.tile([128, NT_B], BF16, tag="scol")
                v_ps = ps_v.tile([1, D], F32, tag="vps")
                for blk in range(NBLK):
                    f0 = blk * 512
                    u_ps = ps_u.tile([1, 512], F32, tag="ups")
                    nc.tensor.matmul(
                        u_ps[:], t_bf[:], Rd_sb[:, f0:f0 + 512],
                        start=True, stop=True,
                    )
                    if blk % 2 == 0:
                        nc.scalar.activation(
                            s_row[:, f0:f0 + 512], u_ps[:], AF.Copy,
                            bias=float(N + 1) / N, scale=-1.0 / N,
                        )
                    else:
                        nc.vector.tensor_scalar(
                            s_row[:, f0:f0 + 512], u_ps[:],
                            -1.0 / N, float(N + 1) / N, OP.mult, OP.add,
                        )
                    s_dram = dpool.tile([1, 512], BF16, tag="sdram")
                    nc.sync.dma_start(s_dram[:], s_row[:, f0:f0 + 512])
                    nc.sync.dma_start(
                        s_col[:, 4 * blk:4 * blk + 4],
                        s_dram.rearrange("a (j p) -> (a p) j", p=128),
                    )
                    # v_mean += s_blk^T @ V rows of this block (4 row-tiles)
                    for j in range(4 * blk, 4 * blk + 4):
                        nc.tensor.matmul(
                            v_ps[:], s_col[:, j:j + 1],
                            vnat[:, j * D:(j + 1) * D],
                            start=(j == 0), stop=(j == NT_B - 1),
                        )
                nc.scalar.activation(
                    out_sb[:, b * D:(b + 1) * D], v_ps[:], AF.Copy,
                )

            nc.sync.dma_start(out_d[:], out_sb[:])

    nc.compile()
    return nc


def _host_prep(inputs, cfg):
    """Weight-norm, packing, casts; returns per-core input maps."""
    def wn(v, g):
        return v * (g / np.linalg.norm(v.astype(np.float64), axis=1)).astype(
            np.float32
        )[:, None]

    W1 = wn(np.asarray(inputs["U1_v"], np.float32), np.asarray(inputs["U1_g"], np.float32))
    W2 = wn(np.asarray(inputs["U2_v"], np.float32), np.asarray(inputs["U2_g"], np.float32))
    # lhsT layout [d, m]: m 0-63 -> R (W1), 64-127 -> L (W2); split d in 2 chunks
    W12T = np.concatenate([W1.T, W2.T], axis=1)          # [D, 128]
    w12 = np.ascontiguousarray(
        W12T.reshape(2, 128, 128)
    ).astype(ml_dtypes.bfloat16)                          # [c, d, m]
    w12 = np.ascontiguousarray(w12.transpose(1, 0, 2))    # [d, c, m]
    b12 = np.concatenate([
        np.asarray(inputs["U1_b"], np.float32),
        np.asarray(inputs["U2_b"], np.float32),
    ]).reshape(128, 1)

    V = np.asarray(inputs["Vmat"], np.float32)  # [B, N, D]
    Vb = V.astype(ml_dtypes.bfloat16)
    in_maps = []
    for k in range(NCORES):
        packs = []
        for b in range(B_LOC):
            Vk = Vb[k * B_LOC + b]                                    # [N, D]
            vt = Vk.T.reshape(2, 128, N).transpose(1, 0, 2).reshape(128, 2 * N)
            vn = Vk.reshape(NT_B, 128, D).transpose(1, 0, 2).reshape(128, NT_B * D)
            packs.append(np.concatenate([vt, vn], axis=1))            # [128, VIN_W]
        vin = np.ascontiguousarray(np.stack(packs, axis=1))           # [128, B_LOC, VIN_W]
        in_maps.append({"vin": vin, "w12": w12, "b12": b12})
    return in_maps


def _epilogue(v_mean, inputs):
    """feat = v_mean @ W_lin.T + b_lin, then training-mode batchnorm."""
    W_lin = np.asarray(inputs["W_lin"], np.float32)
    b_lin = np.asarray(inputs["b_lin"], np.float32)
    gamma = np.asarray(inputs["gamma"], np.float32)
    beta = np.asarray(inputs["beta"], np.float32)
    feat = v_mean.astype(np.float32) @ W_lin.T + b_lin
    mu = feat.mean(axis=0)
    var = feat.var(axis=0)
    out = (feat - mu) / np.sqrt(var + EPS_BN) * gamma + beta
    return out.astype(np.float32)


def kernel(**inputs):
    cfg = dict(CONFIG)
    key = ("v2",)
    if key not in _CACHE:
        _CACHE[key] = _build(cfg)
    nc = _CACHE[key]
    in_maps = _host_prep(inputs, cfg)
    res = run_bass_kernel_spmd(
        nc, in_maps, core_ids=list(range(NCORES)), trace=cfg["trace"]
    )
    kernel.last_results = res
    v_mean = np.concatenate(
        [res.results[k]["vmean"].reshape(B_LOC, D) for k in range(NCORES)], axis=0
    )
    return _epilogue(v_mean, inputs)


# revision 14
# speedup vs baseline: 1.6750x; 1.0597x over previous
"""Trainium2 Bass kernel: Encoder_HieStackedCorr (UnCorrVmat_Detail, t_method='uncorr').

Math (per batch b):
    W1 = wn(U1_v, U1_g); W2 = wn(U2_v, U2_g)
    R = relu(V @ W1.T + b1)          [N, LR]
    L = relu(V @ W2.T + b2)          [N, LR]
    UnCorr = L @ R.T                 [N, N]
    d[n] = UnCorr[n, n] = sum_l L[n,l] R[n,l]
    dr = 1/sqrt(d + eps)
    A = 1 + I - dr dr^T * UnCorr
    v = mean_n (A @ V) = (1/N) * s @ V  where s[m] = N + 1 - dr[m] * (t . R[m,:]),
                                              t = sum_n dr[n] L[n,:]
    feat = v @ W_lin.T + b_lin ; out = batchnorm(feat)   (training-mode stats)

The N x N matrix is never materialized: mean-pooling commutes with the matmul,
collapsing the O(B N^2 (LR+D)) reference into O(B N D LR) work.

Kernel structure (v2):
  - W1 || W2 stacked into one [128, 128] lhsT: a single matmul pair per
    512-block produces both R (partitions 0-63) and L (64-127); one fused
    relu+bias on the whole [128, 512] PSUM tile.
  - Diag via ones[64,64] lhsT: d arrives already broadcast over 64
    partitions ([64, 512] PSUM), so rsqrt runs wide (ACT sqrt + DVE
    reciprocal_approx_fast) and no separate broadcast matmul is needed.
  - t via chained tensor_tensor_reduce (mult+add accumulate across blocks).
  - dr is folded into R (Rd = R * drrep on GpSimd), so the u-matmul yields
    c = dr*u directly and s = (N+1-c)/N comes from one ACT affine per block.
  - s scattered to column layout via a DRAM bounce, then 16 accumulating
    matmuls against natural-layout V give v_mean.

Sharding: data-parallel over batch, 4 batches per core on 8 cores.  The tiny
[32,256] linear + batchnorm epilogue (cross-core batch stats) runs on host.
"""

import os
import sys

import numpy as np

for _p in ("/opt/trn_rl_repo", "/root/.axon_site/_ro/trn_rl_repo"):
    if os.path.isdir(_p) and _p not in sys.path:
        sys.path.insert(0, _p)
        break

import ml_dtypes  # noqa: E402
import concourse.bass as bass  # noqa: E402
import concourse.bacc as bacc  # noqa: E402
import concourse.mybir as mybir  # noqa: E402
import concourse.tile as tile  # noqa: E402
from concourse.bass_utils import run_bass_kernel_spmd  # noqa: E402


def _ensure_ntff_hook():
    """Shim the missing ``antenv.axon_hooks`` registry so trace=True works."""
    import types

    try:
        from antenv.axon_hooks import get_axon_ntff_profile_hook  # noqa: F401
        return
    except ImportError:
        pass
    try:
        from trn_agent_boot.trn_boot import _ntff_profile_via_ctypes
        hook = _ntff_profile_via_ctypes("/opt/axon/libaxon_pjrt.so")
    except Exception:
        hook = None
    mod = types.ModuleType("antenv.axon_hooks")
    mod._hook = hook
    mod.get_axon_ntff_profile_hook = lambda: mod._hook
    mod.set_axon_ntff_profile_hook = lambda h: setattr(mod, "_hook", h)
    sys.modules["antenv.axon_hooks"] = mod


_ensure_ntff_hook()

# Problem constants (hardcoded; see module docstring).
B, N, D, LR, EMB = 32, 2048, 256, 64, 256
NCORES = 8
B_LOC = B // NCORES          # 4 batches per core
NT_B = N // 128              # 16 row-tiles per batch
NBLK = N // 512              # 4 512-col blocks per batch
VIN_W = 2 * N + NT_B * D     # per-batch packed width: vt (2*N) + v-nat (16*D)
EPS_DIAG = 1e-6
EPS_BN = 1e-5

F32 = mybir.dt.float32
BF16 = mybir.dt.bfloat16

CONFIG = dict(dt="bf16", trace=False)

_CACHE = {}


def _build(cfg):
    nc = bacc.Bacc("TRN2", target_bir_lowering=False, debug=False)

    vin_d = nc.dram_tensor("vin", [128, B_LOC, VIN_W], BF16, kind="ExternalInput").ap()
    w12_d = nc.dram_tensor("w12", [128, 2, 128], BF16, kind="ExternalInput").ap()
    b12_d = nc.dram_tensor("b12", [128, 1], F32, kind="ExternalInput").ap()
    out_d = nc.dram_tensor("vmean", [1, B_LOC * D], F32, kind="ExternalOutput").ap()

    AF = mybir.ActivationFunctionType
    OP = mybir.AluOpType

    with tile.TileContext(nc) as tc:
        with (
            tc.tile_pool(name="const", bufs=1) as cpool,
            tc.tile_pool(name="vin", bufs=B_LOC) as vpool,
            tc.tile_pool(name="lr", bufs=3) as lrpool,
            tc.tile_pool(name="rd", bufs=3) as rdpool,
            tc.tile_pool(name="blk", bufs=3) as bpool,
            tc.tile_pool(name="sq", bufs=3) as sqpool,
            tc.tile_pool(name="dr", bufs=3) as drpool,
            tc.tile_pool(name="tsm", bufs=12) as tpool,
            tc.tile_pool(name="srow", bufs=3) as srpool,
            tc.tile_pool(name="scol", bufs=3) as scpool,
            tc.tile_pool(name="ps_lr", bufs=2, space="PSUM") as ps_lr,
            tc.tile_pool(name="ps_d", bufs=2, space="PSUM") as ps_d,
            tc.tile_pool(name="ps_u", bufs=2, space="PSUM") as ps_u,
            tc.tile_pool(name="ps_v", bufs=2, space="PSUM") as ps_v,
            tc.tile_pool(name="dram", bufs=4, space="DRAM") as dpool,
        ):
            # ---- constants / weights ----
            w12_sb = cpool.tile([128, 2 * 128], BF16)
            nc.sync.dma_start(
                w12_sb[:].rearrange("p (c m) -> p c m", c=2), w12_d[:]
            )
            b12_sb = cpool.tile([128, 1], F32)
            nc.sync.dma_start(b12_sb[:], b12_d[:])
            ones64 = cpool.tile([64, 64], BF16)
            nc.vector.memset(ones64[:], 1.0)
            eps64 = cpool.tile([64, 1], F32)
            nc.vector.memset(eps64[:], EPS_DIAG)
            out_sb = cpool.tile([1, B_LOC * D], F32)

            # ---- per-batch inputs: vt halves first (feed main matmuls),
            # natural-V halves after (only needed by the final matmuls) ----
            vin = {}
            for b in range(B_LOC):
                vin[b] = vpool.tile([128, VIN_W], BF16, tag="vin", name=f"vin{b}")
                nc.sync.dma_start(vin[b][:, 0:2 * N], vin_d[:, b, 0:2 * N])
            for b in range(B_LOC):
                nc.sync.dma_start(vin[b][:, 2 * N:], vin_d[:, b, 2 * N:])

            # ---- PE warm-up: ~5us of dummy matmuls during the input-DMA
            # dead time.  The HAM clock gate only lifts (1.2 -> 2.4 GHz)
            # after ~3.4us of SUSTAINED matmul activity; without this the
            # whole kernel runs at half PE clock. ----
            junk = cpool.tile([64, 512], BF16)
            nc.vector.memset(junk[:], 1.0)
            for w in range(16):
                w_ps = ps_d.tile([64, 512], F32, tag="dps")
                nc.tensor.matmul(
                    w_ps[:], ones64[:], junk[:], start=True, stop=True,
                )

            for b in range(B_LOC):
                vt_v = vin[b][:, 0:2 * N]          # [128, c*N+n] transposed V
                vnat = vin[b][:, 2 * N:]           # [128, j*D+dd] natural V

                R_sb = lrpool.tile([64, N], BF16, tag="R", name=f"R{b}")
                L_sb = lrpool.tile([64, N], BF16, tag="L", name=f"L{b}")
                Rd_sb = rdpool.tile([64, N], BF16, tag="Rd")
                ldr_sb = rdpool.tile([64, N], BF16, tag="ldr")
                t_parts = tpool.tile([64, NBLK], F32, tag="tp", name=f"tp{b}")

                for blk in range(NBLK):
                    f0 = blk * 512
                    # [R; L] = W12.T @ vt  (R rows 0-63, L rows 64-127)
                    lr_ps = ps_lr.tile([128, 512], F32, tag="lrps")
                    for c in range(2):
                        nc.tensor.matmul(
                            lr_ps[:],
                            w12_sb[:, c * 128:(c + 1) * 128],
                            vt_v[:, c * N + f0:c * N + f0 + 512],
                            start=(c == 0), stop=(c == 1),
                        )
                    # split relu: R half on ACT, L half on ACT (cross-base
                    # PSUM read is legal); both land at partition base 0.
                    nc.scalar.activation(
                        R_sb[:, f0:f0 + 512], lr_ps[0:64, :], AF.Relu,
                        bias=b12_sb[0:64, :], scale=1.0,
                    )
                    if blk % 2 == 0:
                        nc.scalar.activation(
                            L_sb[:, f0:f0 + 512], lr_ps[64:128, :], AF.Relu,
                            bias=b12_sb[64:128, :], scale=1.0,
                        )
                    else:
                        nc.vector.tensor_scalar(
                            L_sb[:, f0:f0 + 512], lr_ps[64:128, :],
                            b12_sb[64:128, :], 0.0, OP.add, OP.max,
                        )
                    # prod = L * R on DVE (block-chain critical: feeds diag)
                    prod = bpool.tile([64, 512], BF16, tag="prod")
                    nc.vector.tensor_tensor(
                        prod[:], R_sb[:, f0:f0 + 512],
                        L_sb[:, f0:f0 + 512], OP.mult,
                    )
                    # d broadcast over 64 partitions via ones-lhsT matmul
                    d_ps = ps_d.tile([64, 512], F32, tag="dps")
                    nc.tensor.matmul(
                        d_ps[:], ones64[:], prod[:], start=True, stop=True,
                    )
                    # drrep = 1/sqrt(d + eps), computed wide
                    sqrep = sqpool.tile([64, 512], F32, tag="sq")
                    nc.scalar.activation(
                        sqrep[:], d_ps[:], AF.Sqrt, bias=eps64[:], scale=1.0,
                    )
                    drrep = drpool.tile([64, 512], F32, tag="dr")
                    nc.vector.reciprocal_approx_fast(drrep[:], sqrep[:])
                    # ldr = L * drrep (summed below for t); Rd = R * drrep so
                    # the u-matmul yields c = dr*u directly.  Rd has slack
                    # (phase B waits on t anyway) -> idle GpSimd; ldr of the
                    # last block gates t -> keep that one on DVE.
                    if blk == NBLK - 1:
                        nc.vector.tensor_tensor(
                            ldr_sb[:, f0:f0 + 512], L_sb[:, f0:f0 + 512],
                            drrep[:], OP.mult,
                        )
                    else:
                        nc.gpsimd.tensor_tensor(
                            ldr_sb[:, f0:f0 + 512], L_sb[:, f0:f0 + 512],
                            drrep[:], OP.mult,
                        )
                    nc.gpsimd.tensor_tensor(
                        Rd_sb[:, f0:f0 + 512], R_sb[:, f0:f0 + 512],
                        drrep[:], OP.mult,
                    )
                    # partial t for this block (full t = sum of partials)
                    nc.vector.tensor_reduce(
                        t_parts[:, blk:blk + 1], ldr_sb[:, f0:f0 + 512],
                        mybir.AxisListType.X, OP.add,
                    )

                # t = sum_n dr[n] * L[:, n]
                t_f = tpool.tile([64, 1], F32, tag="tf", name=f"tf{b}")
                nc.vector.tensor_reduce(
                    t_f[:], t_parts[:], mybir.AxisListType.X, OP.add,
                )
                t_bf = tpool.tile([64, 1], BF16, tag="tbf", name=f"tbf{b}")
                nc.scalar.activation(t_bf[:], t_f[:], AF.Copy)

                # c = dr * (t . R[m,:]) ; s = ((N+1) - c)/N.  The partition
                # scatter bounces through DRAM per 512-block so the strided
                # gather DMAs and the final matmul groups pipeline.
                s_row = srpool.tile([1, N], BF16, tag="srow")
                s_col = scpool.tile([128, NT_B], BF16, tag="scol")
                v_ps = ps_v.tile([1, D], F32, tag="vps")
                for blk in range(NBLK):
                    f0 = blk * 512
                    u_ps = ps_u.tile([1, 512], F32, tag="ups")
                    nc.tensor.matmul(
                        u_ps[:], t_bf[:], Rd_sb[:, f0:f0 + 512],
                        start=True, stop=True,
                    )
                    if blk % 2 == 0:
                        nc.scalar.activation(
                            s_row[:, f0:f0 + 512], u_ps[:], AF.Copy,
                            bias=float(N + 1) / N, scale=-1.0 / N,
                        )
                    else:
                        nc.vector.tensor_scalar(
                            s_row[:, f0:f0 + 512], u_ps[:],
                            -1.0 / N, float(N + 1) / N, OP.mult, OP.add,
                        )
                    s_dram = dpool.tile([1, 512], BF16, tag="sdram")
                    nc.sync.dma_start(s_dram[:], s_row[:, f0:f0 + 512])
                    nc.sync.dma_start(
                        s_col[:, 4 * blk:4 * blk + 4],
                        s_dram.rearrange("a (j p) -> (a p) j", p=128),
                    )
                    # v_mean += s_blk^T @ V rows of this block (4 row-tiles)
                    for j in range(4 * blk, 4 * blk + 4):
                        nc.tensor.matmul(
                            v_ps[:], s_col[:, j:j + 1],
                            vnat[:, j * D:(j + 1) * D],
                            start=(j == 0), stop=(j == NT_B - 1),
                        )
                nc.scalar.activation(
                    out_sb[:, b * D:(b + 1) * D], v_ps[:], AF.Copy,
                )
                nc.sync.dma_start(
                    out_d[:, b * D:(b + 1) * D], out_sb[:, b * D:(b + 1) * D]
                )

    nc.compile()
    return nc


def _host_prep(inputs, cfg):
    """Weight-norm, packing, casts; returns per-core input maps."""
    def wn(v, g):
        return v * (g / np.linalg.norm(v.astype(np.float64), axis=1)).astype(
            np.float32
        )[:, None]

    W1 = wn(np.asarray(inputs["U1_v"], np.float32), np.asarray(inputs["U1_g"], np.float32))
    W2 = wn(np.asarray(inputs["U2_v"], np.float32), np.asarray(inputs["U2_g"], np.float32))
    # lhsT layout [d, m]: m 0-63 -> R (W1), 64-127 -> L (W2); split d in 2 chunks
    W12T = np.concatenate([W1.T, W2.T], axis=1)          # [D, 128]
    w12 = np.ascontiguousarray(
        W12T.reshape(2, 128, 128)
    ).astype(ml_dtypes.bfloat16)                          # [c, d, m]
    w12 = np.ascontiguousarray(w12.transpose(1, 0, 2))    # [d, c, m]
    b12 = np.concatenate([
        np.asarray(inputs["U1_b"], np.float32),
        np.asarray(inputs["U2_b"], np.float32),
    ]).reshape(128, 1)

    V = np.asarray(inputs["Vmat"], np.float32)  # [B, N, D]
    Vb = V.astype(ml_dtypes.bfloat16)
    in_maps = []
    for k in range(NCORES):
        packs = []
        for b in range(B_LOC):
            Vk = Vb[k * B_LOC + b]                                    # [N, D]
            vt = Vk.T.reshape(2, 128, N).transpose(1, 0, 2).reshape(128, 2 * N)
            vn = Vk.reshape(NT_B, 128, D).transpose(1, 0, 2).reshape(128, NT_B * D)
            packs.append(np.concatenate([vt, vn], axis=1))            # [128, VIN_W]
        vin = np.ascontiguousarray(np.stack(packs, axis=1))           # [128, B_LOC, VIN_W]
        in_maps.append({"vin": vin, "w12": w12, "b12": b12})
    return in_maps


def _epilogue(v_mean, inputs):
    """feat = v_mean @ W_lin.T + b_lin, then training-mode batchnorm."""
    W_lin = np.asarray(inputs["W_lin"], np.float32)
    b_lin = np.asarray(inputs["b_lin"], np.float32)
    gamma = np.asarray(inputs["gamma"], np.float32)
    beta = np.asarray(inputs["beta"], np.float32)
    feat = v_mean.astype(np.float32) @ W_lin.T + b_lin
    mu = feat.mean(axis=0)
    var = feat.var(axis=0)
    out = (feat - mu) / np.sqrt(var + EPS_BN) * gamma + beta
    return out.astype(np.float32)


def kernel(**inputs):
    cfg = dict(CONFIG)
    key = ("v2",)
    if key not in _CACHE:
        _CACHE[key] = _build(cfg)
    nc = _CACHE[key]
    in_maps = _host_prep(inputs, cfg)
    res = run_bass_kernel_spmd(
        nc, in_maps, core_ids=list(range(NCORES)), trace=cfg["trace"]
    )
    kernel.last_results = res
    v_mean = np.concatenate(
        [res.results[k]["vmean"].reshape(B_LOC, D) for k in range(NCORES)], axis=0
    )
    return _epilogue(v_mean, inputs)
